# revision 28
# baseline (speedup 1.0000x reference)
"""Trainium2 Bass kernel for nn_CASCADES_v8_ResonantCore (moe_routing).

Computation (per batch b):
    centroid = 0.7*x[b,-1,:] + 0.3*mean_s(x[b])
    w = softmax(cos_sim(centroid, core_keys)/TEMP)      # [K]
    Lam = sum_k w[k] * core_pool[k]                     # [R,R]
    out[b] = ((x[b] @ V^T) @ Lam^T) @ U^T               # [S,D]

Strategy (8 cores, data-parallel over (batch, seq-half)):
  - K1 (read pass): each core reads its [2048, 4096] shard of x once via
    2MB cast-DMAs (f32 HBM -> bf16 SBUF), computes xV^T = V @ x^T on PE
    (transpose chunks + 4-way col-packed matmuls, fp32 accumulate) and
    the column sums of x (bf16 strip accumulation on Vector + one final
    cross-partition PE reduce).
  - Host: combines the 8 partial column sums, does the tiny routing math
    (cosine/softmax over 16 numbers), folds Lam into W = U @ Lam.
  - K2 (write pass): each core computes out = xV @ W^T and writes its
    [2048, 4096] shard of the output in bf16 (host upcasts to f32; the
    harness tolerance is 2e-2 relative-to-max, bf16 rounding is ~4e-3).
  Total HBM traffic is read-x (f32, mandatory) + write-out (bf16); the
  two passes are inherently serial because every output element depends
  on the full-sequence mean through the routing weights.
"""

import sys

sys.path.insert(0, "/opt/trn_rl_repo")

import contextlib

import ml_dtypes
import numpy as np

import concourse.bass as bass  # noqa: F401  (registers bass types)
import concourse.tile as tile
from concourse import bacc, mybir
from concourse.bass_utils import run_bass_kernel_spmd

BF16 = ml_dtypes.bfloat16

B, S, D, R, K = 4, 4096, 4096, 8, 4
NCORES = 8
SH = S // 2  # rows of x per core
EPS, TEMP = 1e-8, 0.05

_cache = {}


def build_k1(sh=SH, d=D, r=R):
    """Read pass: xs [sh, d] f32 -> xvt [4r, sh] f32 (4 col-bands of V @ x^T),
    cs [1, d] f32 (column sums of bf16(x))."""
    nstrip, nch = sh // 128, d // 128  # 16 strips, 32 chunks/strip
    ngrp = nch // 8  # 4 transpose groups of 8 chunks -> [128, 1024] psum
    nc = bacc.Bacc("TRN2", target_bir_lowering=False, debug=False)
    xs = nc.dram_tensor("xs", [sh, d], mybir.dt.float32, kind="ExternalInput").ap()
    vt = nc.dram_tensor("vt", [128, nch * r], mybir.dt.bfloat16, kind="ExternalInput").ap()
    consts = nc.dram_tensor("consts", [128, 768], mybir.dt.bfloat16, kind="ExternalInput").ap()
    xvt_out = nc.dram_tensor("xvt", [4 * r, sh], mybir.dt.float32, kind="ExternalOutput").ap()
    cs_out = nc.dram_tensor("cs", [1, d], mybir.dt.float32, kind="ExternalOutput").ap()

    with tile.TileContext(nc) as tc:
        with contextlib.ExitStack() as ctx:
            cpool = ctx.enter_context(tc.tile_pool(name="consts", bufs=1))
            xpool = ctx.enter_context(tc.tile_pool(name="x", bufs=4))
            xspool = ctx.enter_context(tc.tile_pool(name="xsp", bufs=1))
            tpool = ctx.enter_context(tc.tile_pool(name="xT8", bufs=4))
            apool = ctx.enter_context(tc.tile_pool(name="acc", bufs=1))
            opool = ctx.enter_context(tc.tile_pool(name="outs", bufs=1))
            psT = ctx.enter_context(tc.tile_pool(name="psT", bufs=2, space="PSUM"))
            psX = ctx.enter_context(tc.tile_pool(name="psX", bufs=2, space="PSUM"))
            psR = ctx.enter_context(tc.tile_pool(name="psR", bufs=2, space="PSUM"))

            const_sb = cpool.tile([128, 768], mybir.dt.bfloat16)
            nc.sync.dma_start(const_sb[:], consts[:])
            vt_sb = cpool.tile([128, nch * r], mybir.dt.bfloat16)
            nc.sync.dma_start(vt_sb[:], vt[:])
            idn = const_sb[:, 0:128]
            ones = const_sb[:, 128:129]
            dummy_rhs = const_sb[:, 256:768]
            # 4 row-bands of r at partitions {0,32,64,96}; host sums the bands
            xvt_sb = opool.tile([128, sh], mybir.dt.float32)
            cs_sb = opool.tile([1, d], mybir.dt.float32)
            acc = apool.tile([128, d], mybir.dt.bfloat16)

            def warm_mm(rhs, w=512):
                # throwaway matmul that keeps the PE HAM activity monitor
                # busy (cold PE runs at 1.2 GHz instead of 2.4)
                pw = psR.tile([1, 512], mybir.dt.float32, tag="psR")
                nc.tensor.matmul(pw[:, 0:w], ones, rhs, start=True, stop=True,
                                 skip_group_check=True)

            # warm-up burst: spans the NEFF-startup window so strip 0's
            # transposes run at full clock
            for _ in range(10):
                warm_mm(dummy_rhs)

            # strip -> list of (col_lo, piece-width, tile); strip 0 arrives
            # in 4 pieces (PE gets work during the warm-up ramp), strip 15 in
            # 2 (the final colsum add starts earlier), the rest as single 2MB
            # DMAs (larger transfers have measurably better per-byte rate)
            def piece_widths(i):
                if i == 0:
                    return [1024] * 4
                if i in (1, 2):
                    # bridge the quarter-cadence -> full-cadence transition:
                    # with strip 1 as one 2MB DMA the PE drains strip 0's
                    # quarters early and idles >3.4us before strip 1 lands,
                    # taking the first HAM re-throttle that seeds the
                    # cold/backlog cycle
                    return [2048] * 2
                return [2048] * 2 if i == nstrip - 1 else [4096]

            xqs = {}
            def issue_strip(i):
                lo = 0
                ps = []
                for pi, w in enumerate(piece_widths(i)):
                    pool = xpool if len(piece_widths(i)) == 1 else xspool
                    xq = pool.tile([128, w], mybir.dt.bfloat16, tag=f"xq{pi}w{w}")
                    nc.gpsimd.dma_start(
                        xq[:], xs[i * 128:(i + 1) * 128, lo:lo + w]
                    )
                    ps.append((lo, w, xq))
                    lo += w
                xqs[i] = ps

            PREFETCH = 3
            for i in range(min(PREFETCH, nstrip)):
                issue_strip(i)

            def mm1(ps, q, p, xT8):
                # one pack: 4 concurrent col-group matmuls, band k takes
                # chunk c = 4p+k (all in xT8 tile p//2); accumulate over
                # p=0..7 into col-range q of the quad's psum tile.
                for k in range(4):
                    c = 4 * p + k
                    cc = c % 8
                    nc.tensor.matmul(
                        ps[32 * k:32 * k + r, q * 128:(q + 1) * 128],
                        vt_sb[:, c * r:(c + 1) * r],
                        xT8[:, cc * 128:(cc + 1) * 128],
                        start=(p == 0),
                        stop=(p == 2 * ngrp - 1),
                        tile_position=(0, 32 * k),
                    )

            def final_reduce():
                # colsum cross-partition reduce on PE, pipelined with
                # per-chunk copies alternating vector/scalar; DMA out in
                # halves so the first write's completion latency overlaps
                # the second
                for j in range(d // 512):
                    psRt = psR.tile([1, 512], mybir.dt.float32, tag="psR")
                    nc.tensor.matmul(
                        psRt[:], ones, acc[:, j * 512:(j + 1) * 512],
                        start=True, stop=True,
                    )
                    dst = cs_sb[:, j * 512:(j + 1) * 512]
                    if j % 2 == 0:
                        nc.vector.tensor_copy(dst, psRt[:])
                    else:
                        nc.scalar.copy(dst, psRt[:])
                    if j == d // 1024 - 1:
                        nc.sync.dma_start(cs_out[:, :d // 2], cs_sb[:, :d // 2])
                nc.sync.dma_start(cs_out[:, d // 2:], cs_sb[:, d // 2:])

            pend = []  # deferred mm1 packs: software-pipeline skew on PE queue
            ps_xvt = None
            for i in range(nstrip):
                pieces = xqs[i]
                if i + PREFETCH < nstrip:
                    issue_strip(i + PREFETCH)
                def adds():
                    # colsum strip accumulation (strip 0 initializes acc)
                    for lo, w, xq in pieces:
                        if i == 0:
                            nc.vector.tensor_copy(acc[:, lo:lo + w], xq[:])
                        else:
                            nc.vector.tensor_add(
                                acc[:, lo:lo + w], acc[:, lo:lo + w], xq[:]
                            )
                if i == nstrip - 1:
                    adds()  # last strip: adds head the queue (tail critical)
                if i % 4 == 0:
                    ps_xvt = psX.tile([128, 512], mybir.dt.float32, tag="psxvt")
                strip_xT8 = []
                for g in range(ngrp):
                    psTt = psT.tile([128, 1024], mybir.dt.float32, tag="psT")
                    for k in range(8):
                        col = (8 * g + k) * 128
                        lo, w, xq = next(
                            pc for pc in pieces if pc[0] <= col < pc[0] + pc[1]
                        )
                        nc.tensor.matmul(
                            psTt[:, k * 128:(k + 1) * 128],
                            xq[:, col - lo:col - lo + 128], idn,
                            start=True, stop=True,
                        )
                    # drain deferred mm1 packs, keeping a 2-group skew so PE
                    # never waits on a just-issued copy
                    while len(pend) > 2:
                        pend.pop(0)()
                    xT8 = tpool.tile([128, 1024], mybir.dt.bfloat16, tag="xT8")
                    # all psT casts on scalar: the vector queue (adds) must
                    # never gate the psT chain the PE transposes wait on
                    nc.scalar.copy(xT8[:], psTt[:])
                    strip_xT8.append(xT8)
                    for p in (2 * g, 2 * g + 1):
                        pend.append(
                            lambda ps=ps_xvt, q=i % 4, p=p, t=xT8: mm1(ps, q, p, t)
                        )
                if i != nstrip - 1:
                    # adds go behind the casts so they never gate the psT
                    # copy chain that the PE transposes wait on
                    adds()
                if i != nstrip - 1:
                    # keep-warm matmuls staggered inside the PE idle window
                    # (waiting for the next strip's DMA), gated on this
                    # strip's add (~+2.3us) and casts g1/g2/g3 (~+3/+4/+5us):
                    # no idle span can then cover the free-running ~3.4us HAM
                    # window, so the PE clock stays at 2.4 GHz
                    # N=64 keeps each dummy ~50ns: when the PE is cold it
                    # must not add real work (a cold N=512 dummy costs 427ns
                    # and 4 of them push the strip over the DMA cadence,
                    # accumulating backlog)
                    warm_mm(acc[:, 0:64], 64)
                    warm_mm(strip_xT8[1][:, 0:64], 64)
                    warm_mm(strip_xT8[2][:, 0:64], 64)
                    warm_mm(strip_xT8[3][:, 0:64], 64)
                else:
                    final_reduce()
                if i % 4 == 3:
                    # quad complete once its mm1 packs retire: flush the skew
                    while pend:
                        pend.pop(0)()
                    lo, hi = (i - 3) * 128, (i + 1) * 128
                    for k in range(4):
                        dst = xvt_sb[32 * k:32 * k + r, lo:hi]
                        nc.vector.tensor_copy(dst, ps_xvt[32 * k:32 * k + r, :])
                    for k in range(4):
                        nc.scalar.dma_start(
                            xvt_out[r * k:r * (k + 1), lo:hi],
                            xvt_sb[32 * k:32 * k + r, lo:hi],
                        )


    nc.compile()
    return nc


def build_k2(sh=SH, d=D, r=R):
    """Write pass: out [sh, d] bf16 = xvt^T @ wt with xvt [r, sh], wt [r, d] bf16."""
    nsx = sh // 128
    nc = bacc.Bacc("TRN2", target_bir_lowering=False, debug=False)
    xvt = nc.dram_tensor("xvt", [r, sh], mybir.dt.bfloat16, kind="ExternalInput").ap()
    wt = nc.dram_tensor("wt", [r, d], mybir.dt.bfloat16, kind="ExternalInput").ap()
    out = nc.dram_tensor("out", [sh, d], mybir.dt.bfloat16, kind="ExternalOutput").ap()

    with tile.TileContext(nc) as tc:
        with contextlib.ExitStack() as ctx:
            cpool = ctx.enter_context(tc.tile_pool(name="consts", bufs=1))
            opool = ctx.enter_context(tc.tile_pool(name="ob", bufs=4))
            psP = ctx.enter_context(tc.tile_pool(name="ps2", bufs=4, space="PSUM"))

            wt_sb = cpool.tile([r, d], mybir.dt.bfloat16)
            xvt_sb = cpool.tile([r, sh], mybir.dt.bfloat16)
            # head chunks first (tiny transfers, completion-latency bound):
            # strip 0's first matmuls need only wt[:, :1024] and xvt[:, :128]
            nc.sync.dma_start(wt_sb[:, 0:1024], wt[:, 0:1024])
            nc.scalar.dma_start(xvt_sb[:, 0:128], xvt[:, 0:128])
            nc.sync.dma_start(wt_sb[:, 1024:], wt[:, 1024:])
            nc.scalar.dma_start(xvt_sb[:, 128:], xvt[:, 128:])

            for i in range(nsx):
                ob = opool.tile([128, d], mybir.dt.bfloat16, tag="ob")
                for j in range(d // 1024):
                    ps2 = psP.tile([128, 1024], mybir.dt.float32, tag="ps2")
                    for half in range(2):
                        nc.tensor.matmul(
                            ps2[:, half * 512:(half + 1) * 512],
                            xvt_sb[:, i * 128:(i + 1) * 128],
                            wt_sb[:, (2 * j + half) * 512:(2 * j + half + 1) * 512],
                            start=True, stop=True,
                        )
                    dst = ob[:, j * 1024:(j + 1) * 1024]
                    if j % 2 == 0:
                        nc.vector.tensor_copy(dst, ps2[:])
                    else:
                        nc.scalar.copy(dst, ps2[:])
                    if i == 0:
                        # first strip: stream each quarter out as soon as its
                        # copy lands (shortens the cold-start ramp)
                        nc.sync.dma_start(
                            out[0:128, j * 1024:(j + 1) * 1024],
                            ob[:, j * 1024:(j + 1) * 1024],
                        )
                    elif j == 1:
                        # stream each strip's first half out early: the DMA
                        # only waits on two copies, halving the per-strip
                        # issue bubble
                        nc.sync.dma_start(
                            out[i * 128:(i + 1) * 128, :d // 2], ob[:, :d // 2]
                        )
                if i > 0:
                    nc.sync.dma_start(
                        out[i * 128:(i + 1) * 128, d // 2:], ob[:, d // 2:]
                    )
    nc.compile()
    return nc


def _get_kernels():
    if "k1" not in _cache:
        _cache["k1"] = build_k1()
        _cache["k2"] = build_k2()
    return _cache["k1"], _cache["k2"]


def _vt_layout(V, d, r):
    """[128, (d//128)*r] bf16 with vt[p, c*r + j] = V[j, c*128 + p]."""
    nch = d // 128
    # V [r, d] -> [r, nch, 128] -> [128, nch, r]
    return np.ascontiguousarray(
        V.reshape(r, nch, 128).transpose(2, 1, 0).reshape(128, nch * r)
    ).astype(BF16)


def _consts_layout():
    c = np.zeros((128, 768), dtype=np.float32)
    c[:, 0:128] = np.eye(128, dtype=np.float32)
    c[:, 128] = 1.0
    return c.astype(BF16)


def _host_routing(x, core_keys, core_pool, U_shared, cs):
    """cs: list of 8 per-core colsum vectors [d] (f64). Returns per-batch
    wt = (U @ Lam_b)^T as bf16 [R, D]."""
    wt_b = []
    for b in range(B):
        colsum = cs[2 * b] + cs[2 * b + 1]
        mean = colsum / S
        centroid = 0.7 * x[b, -1, :].astype(np.float64) + 0.3 * mean
        c_n = centroid / max(np.linalg.norm(centroid), EPS)
        kk = core_keys.astype(np.float64)
        k_n = kk / np.maximum(np.linalg.norm(kk, axis=-1, keepdims=True), EPS)
        sim = c_n @ k_n.T  # [K]
        logits = sim / TEMP
        e = np.exp(logits - logits.max())
        w = e / e.sum()
        Lam = np.einsum("k,kij->ij", w, core_pool.astype(np.float64))  # [R, R]
        W = U_shared.astype(np.float64) @ Lam  # [D, R]
        wt_b.append(np.ascontiguousarray(W.T).astype(BF16))  # [R, D]
    return wt_b


def kernel(x, V_shared, U_shared, core_pool, core_keys):
    x = np.asarray(x)
    V_shared = np.asarray(V_shared)
    U_shared = np.asarray(U_shared)
    core_pool = np.asarray(core_pool)
    core_keys = np.asarray(core_keys)

    nc1, nc2 = _get_kernels()
    core_ids = list(range(NCORES))

    vt_np = _vt_layout(V_shared.astype(np.float32), D, R)
    consts_np = _consts_layout()

    in_maps1 = []
    for c in core_ids:
        b, h = c // 2, c % 2
        xs = np.ascontiguousarray(x[b, h * SH:(h + 1) * SH, :], dtype=np.float32)
        in_maps1.append({"xs": xs, "vt": vt_np, "consts": consts_np})
    res1 = run_bass_kernel_spmd(nc1, in_maps1, core_ids).results

    # --- host routing (tiny: 16 numbers through softmax) ---
    cs = [res1[c]["cs"].astype(np.float64).ravel() for c in core_ids]  # [d]
    # xvt comes back as 4 col-bands of r rows; sum them
    xvt = [
        res1[c]["xvt"].reshape(4, R, SH).sum(axis=0).astype(np.float32)
        for c in core_ids
    ]  # [r, SH]

    wt_b = _host_routing(x, core_keys, core_pool, U_shared, cs)

    in_maps2 = []
    for c in core_ids:
        b = c // 2
        in_maps2.append({"xvt": xvt[c].astype(BF16), "wt": wt_b[b]})
    res2 = run_bass_kernel_spmd(nc2, in_maps2, core_ids).results

    out = np.empty((B, S, D), dtype=np.float32)
    for c in core_ids:
        b, h = c // 2, c % 2
        out[b, h * SH:(h + 1) * SH, :] = res2[c]["out"].astype(np.float32)
    return out


# revision 30
# speedup vs baseline: 1.0018x; 1.0018x over previous
"""Trainium2 Bass kernel for nn_CASCADES_v8_ResonantCore (moe_routing).

Computation (per batch b):
    centroid = 0.7*x[b,-1,:] + 0.3*mean_s(x[b])
    w = softmax(cos_sim(centroid, core_keys)/TEMP)      # [K]
    Lam = sum_k w[k] * core_pool[k]                     # [R,R]
    out[b] = ((x[b] @ V^T) @ Lam^T) @ U^T               # [S,D]

Strategy (8 cores, data-parallel over (batch, seq-half)):
  - K1 (read pass): each core reads its [2048, 4096] shard of x once via
    2MB cast-DMAs (f32 HBM -> bf16 SBUF), computes xV^T = V @ x^T on PE
    (transpose chunks + 4-way col-packed matmuls, fp32 accumulate) and
    the column sums of x (bf16 strip accumulation on Vector + one final
    cross-partition PE reduce).
  - Host: combines the 8 partial column sums, does the tiny routing math
    (cosine/softmax over 16 numbers), folds Lam into W = U @ Lam.
  - K2 (write pass): each core computes out = xV @ W^T and writes its
    [2048, 4096] shard of the output in bf16 (host upcasts to f32; the
    harness tolerance is 2e-2 relative-to-max, bf16 rounding is ~4e-3).
  Total HBM traffic is read-x (f32, mandatory) + write-out (bf16); the
  two passes are inherently serial because every output element depends
  on the full-sequence mean through the routing weights.
"""

import sys

sys.path.insert(0, "/opt/trn_rl_repo")

import contextlib

import ml_dtypes
import numpy as np

import concourse.bass as bass  # noqa: F401  (registers bass types)
import concourse.tile as tile
from concourse import bacc, mybir
from concourse.bass_utils import run_bass_kernel_spmd

BF16 = ml_dtypes.bfloat16

B, S, D, R, K = 4, 4096, 4096, 8, 4
NCORES = 8
SH = S // 2  # rows of x per core
EPS, TEMP = 1e-8, 0.05

_cache = {}


def build_k1(sh=SH, d=D, r=R):
    """Read pass: xs [sh, d] f32 -> xvt [4r, sh] f32 (4 col-bands of V @ x^T),
    cs [1, d] f32 (column sums of bf16(x))."""
    nstrip, nch = sh // 128, d // 128  # 16 strips, 32 chunks/strip
    ngrp = nch // 8  # 4 transpose groups of 8 chunks -> [128, 1024] psum
    nc = bacc.Bacc("TRN2", target_bir_lowering=False, debug=False)
    xs = nc.dram_tensor("xs", [sh, d], mybir.dt.float32, kind="ExternalInput").ap()
    vt = nc.dram_tensor("vt", [128, nch * r], mybir.dt.bfloat16, kind="ExternalInput").ap()
    consts = nc.dram_tensor("consts", [128, 768], mybir.dt.bfloat16, kind="ExternalInput").ap()
    xvt_out = nc.dram_tensor("xvt", [4 * r, sh], mybir.dt.float32, kind="ExternalOutput").ap()
    cs_out = nc.dram_tensor("cs", [1, d], mybir.dt.float32, kind="ExternalOutput").ap()

    with tile.TileContext(nc) as tc:
        with contextlib.ExitStack() as ctx:
            cpool = ctx.enter_context(tc.tile_pool(name="consts", bufs=1))
            xpool = ctx.enter_context(tc.tile_pool(name="x", bufs=4))
            xspool = ctx.enter_context(tc.tile_pool(name="xsp", bufs=1))
            xbpool = ctx.enter_context(tc.tile_pool(name="xb", bufs=2))
            tpool = ctx.enter_context(tc.tile_pool(name="xT8", bufs=4))
            apool = ctx.enter_context(tc.tile_pool(name="acc", bufs=1))
            opool = ctx.enter_context(tc.tile_pool(name="outs", bufs=1))
            psT = ctx.enter_context(tc.tile_pool(name="psT", bufs=2, space="PSUM"))
            psX = ctx.enter_context(tc.tile_pool(name="psX", bufs=2, space="PSUM"))
            psR = ctx.enter_context(tc.tile_pool(name="psR", bufs=2, space="PSUM"))

            const_sb = cpool.tile([128, 768], mybir.dt.bfloat16)
            nc.sync.dma_start(const_sb[:], consts[:])
            vt_sb = cpool.tile([128, nch * r], mybir.dt.bfloat16)
            nc.sync.dma_start(vt_sb[:], vt[:])
            idn = const_sb[:, 0:128]
            ones = const_sb[:, 128:129]
            dummy_rhs = const_sb[:, 256:768]
            # 4 row-bands of r at partitions {0,32,64,96}; host sums the bands
            xvt_sb = opool.tile([128, sh], mybir.dt.float32)
            cs_sb = opool.tile([1, d], mybir.dt.float32)
            acc = apool.tile([128, d], mybir.dt.bfloat16)

            def warm_mm(rhs, w=512):
                # throwaway matmul that keeps the PE HAM activity monitor
                # busy (cold PE runs at 1.2 GHz instead of 2.4)
                pw = psR.tile([1, 512], mybir.dt.float32, tag="psR")
                nc.tensor.matmul(pw[:, 0:w], ones, rhs, start=True, stop=True,
                                 skip_group_check=True)

            # warm-up burst: spans the NEFF-startup window so strip 0's
            # transposes run at full clock
            for _ in range(10):
                warm_mm(dummy_rhs)

            # strip -> list of (col_lo, piece-width, tile); strip 0 arrives
            # in 4 pieces (PE gets work during the warm-up ramp), strip 15 in
            # 2 (the final colsum add starts earlier), the rest as single 2MB
            # DMAs (larger transfers have measurably better per-byte rate)
            def piece_widths(i):
                if i == 0:
                    return [1024] * 4
                if i in (1, 2) or i == nstrip - 1:
                    # strips 1-2 bridge the quarter-cadence -> full-cadence
                    # transition (else the PE drains strip 0's quarters
                    # early, idles >3.4us before strip 1 lands, and takes
                    # the first HAM re-throttle that seeds the cold/backlog
                    # cycle); strip 15's halves start the final colsum add
                    # earlier. These 2-piece strips live in their own
                    # double-buffered pool so consecutive users never
                    # serialize on tile reuse.
                    return [2048] * 2
                return [4096]

            xqs = {}
            def issue_strip(i):
                lo = 0
                ps = []
                widths = piece_widths(i)
                for pi, w in enumerate(widths):
                    if len(widths) == 1:
                        pool = xpool
                    elif i == 0:
                        pool = xspool
                    else:
                        pool = xbpool
                    xq = pool.tile([128, w], mybir.dt.bfloat16, tag=f"xq{pi}w{w}")
                    nc.gpsimd.dma_start(
                        xq[:], xs[i * 128:(i + 1) * 128, lo:lo + w]
                    )
                    ps.append((lo, w, xq))
                    lo += w
                xqs[i] = ps

            PREFETCH = 3
            for i in range(min(PREFETCH, nstrip)):
                issue_strip(i)

            def mm1(ps, q, p, xT8):
                # one pack: 4 concurrent col-group matmuls, band k takes
                # chunk c = 4p+k (all in xT8 tile p//2); accumulate over
                # p=0..7 into col-range q of the quad's psum tile.
                for k in range(4):
                    c = 4 * p + k
                    cc = c % 8
                    nc.tensor.matmul(
                        ps[32 * k:32 * k + r, q * 128:(q + 1) * 128],
                        vt_sb[:, c * r:(c + 1) * r],
                        xT8[:, cc * 128:(cc + 1) * 128],
                        start=(p == 0),
                        stop=(p == 2 * ngrp - 1),
                        tile_position=(0, 32 * k),
                    )

            def final_reduce():
                # colsum cross-partition reduce on PE, pipelined with
                # per-chunk copies alternating vector/scalar; DMA out in
                # halves so the first write's completion latency overlaps
                # the second
                for j in range(d // 512):
                    psRt = psR.tile([1, 512], mybir.dt.float32, tag="psR")
                    nc.tensor.matmul(
                        psRt[:], ones, acc[:, j * 512:(j + 1) * 512],
                        start=True, stop=True,
                    )
                    dst = cs_sb[:, j * 512:(j + 1) * 512]
                    if j % 2 == 0:
                        nc.vector.tensor_copy(dst, psRt[:])
                    else:
                        nc.scalar.copy(dst, psRt[:])
                    if j == d // 1024 - 1:
                        nc.sync.dma_start(cs_out[:, :d // 2], cs_sb[:, :d // 2])
                nc.sync.dma_start(cs_out[:, d // 2:], cs_sb[:, d // 2:])

            pend = []  # deferred mm1 packs: software-pipeline skew on PE queue
            ps_xvt = None
            for i in range(nstrip):
                pieces = xqs[i]
                if i + PREFETCH < nstrip:
                    issue_strip(i + PREFETCH)
                def adds():
                    # colsum strip accumulation (strip 0 initializes acc)
                    for lo, w, xq in pieces:
                        if i == 0:
                            nc.vector.tensor_copy(acc[:, lo:lo + w], xq[:])
                        else:
                            nc.vector.tensor_add(
                                acc[:, lo:lo + w], acc[:, lo:lo + w], xq[:]
                            )
                if i == nstrip - 1:
                    adds()  # last strip: adds head the queue (tail critical)
                if i % 4 == 0:
                    ps_xvt = psX.tile([128, 512], mybir.dt.float32, tag="psxvt")
                strip_xT8 = []
                for g in range(ngrp):
                    psTt = psT.tile([128, 1024], mybir.dt.float32, tag="psT")
                    for k in range(8):
                        col = (8 * g + k) * 128
                        lo, w, xq = next(
                            pc for pc in pieces if pc[0] <= col < pc[0] + pc[1]
                        )
                        nc.tensor.matmul(
                            psTt[:, k * 128:(k + 1) * 128],
                            xq[:, col - lo:col - lo + 128], idn,
                            start=True, stop=True,
                        )
                    # drain deferred mm1 packs, keeping a 2-group skew so PE
                    # never waits on a just-issued copy
                    while len(pend) > 2:
                        pend.pop(0)()
                    xT8 = tpool.tile([128, 1024], mybir.dt.bfloat16, tag="xT8")
                    # all psT casts on scalar: the vector queue (adds) must
                    # never gate the psT chain the PE transposes wait on
                    nc.scalar.copy(xT8[:], psTt[:])
                    strip_xT8.append(xT8)
                    for p in (2 * g, 2 * g + 1):
                        pend.append(
                            lambda ps=ps_xvt, q=i % 4, p=p, t=xT8: mm1(ps, q, p, t)
                        )
                if i != nstrip - 1:
                    # adds go behind the casts so they never gate the psT
                    # copy chain that the PE transposes wait on
                    adds()
                if i != nstrip - 1:
                    # keep-warm matmuls staggered inside the PE idle window
                    # (waiting for the next strip's DMA), gated on this
                    # strip's add (~+2.3us) and casts g1/g2/g3 (~+3/+4/+5us):
                    # no idle span can then cover the free-running ~3.4us HAM
                    # window, so the PE clock stays at 2.4 GHz
                    # N=64 keeps each dummy ~50ns: when the PE is cold it
                    # must not add real work (a cold N=512 dummy costs 427ns
                    # and 4 of them push the strip over the DMA cadence,
                    # accumulating backlog)
                    warm_mm(acc[:, 0:64], 64)
                    warm_mm(strip_xT8[1][:, 0:64], 64)
                    warm_mm(strip_xT8[2][:, 0:64], 64)
                    warm_mm(strip_xT8[3][:, 0:64], 64)
                else:
                    final_reduce()
                if i % 4 == 3:
                    # quad complete once its mm1 packs retire: flush the skew
                    while pend:
                        pend.pop(0)()
                    lo, hi = (i - 3) * 128, (i + 1) * 128
                    for k in range(4):
                        dst = xvt_sb[32 * k:32 * k + r, lo:hi]
                        nc.vector.tensor_copy(dst, ps_xvt[32 * k:32 * k + r, :])
                    for k in range(4):
                        nc.scalar.dma_start(
                            xvt_out[r * k:r * (k + 1), lo:hi],
                            xvt_sb[32 * k:32 * k + r, lo:hi],
                        )


    nc.compile()
    return nc


def build_k2(sh=SH, d=D, r=R):
    """Write pass: out [sh, d] bf16 = xvt^T @ wt with xvt [r, sh], wt [r, d] bf16."""
    nsx = sh // 128
    nc = bacc.Bacc("TRN2", target_bir_lowering=False, debug=False)
    xvt = nc.dram_tensor("xvt", [r, sh], mybir.dt.bfloat16, kind="ExternalInput").ap()
    wt = nc.dram_tensor("wt", [r, d], mybir.dt.bfloat16, kind="ExternalInput").ap()
    out = nc.dram_tensor("out", [sh, d], mybir.dt.bfloat16, kind="ExternalOutput").ap()

    with tile.TileContext(nc) as tc:
        with contextlib.ExitStack() as ctx:
            cpool = ctx.enter_context(tc.tile_pool(name="consts", bufs=1))
            opool = ctx.enter_context(tc.tile_pool(name="ob", bufs=4))
            psP = ctx.enter_context(tc.tile_pool(name="ps2", bufs=4, space="PSUM"))

            wt_sb = cpool.tile([r, d], mybir.dt.bfloat16)
            xvt_sb = cpool.tile([r, sh], mybir.dt.bfloat16)
            # head chunks first (tiny transfers, completion-latency bound):
            # strip 0's first matmuls need only wt[:, :1024] and xvt[:, :128]
            nc.sync.dma_start(wt_sb[:, 0:1024], wt[:, 0:1024])
            nc.scalar.dma_start(xvt_sb[:, 0:128], xvt[:, 0:128])
            nc.sync.dma_start(wt_sb[:, 1024:], wt[:, 1024:])
            nc.scalar.dma_start(xvt_sb[:, 128:], xvt[:, 128:])

            for i in range(nsx):
                ob = opool.tile([128, d], mybir.dt.bfloat16, tag="ob")
                for j in range(d // 1024):
                    ps2 = psP.tile([128, 1024], mybir.dt.float32, tag="ps2")
                    for half in range(2):
                        nc.tensor.matmul(
                            ps2[:, half * 512:(half + 1) * 512],
                            xvt_sb[:, i * 128:(i + 1) * 128],
                            wt_sb[:, (2 * j + half) * 512:(2 * j + half + 1) * 512],
                            start=True, stop=True,
                        )
                    dst = ob[:, j * 1024:(j + 1) * 1024]
                    if j % 2 == 0:
                        nc.vector.tensor_copy(dst, ps2[:])
                    else:
                        nc.scalar.copy(dst, ps2[:])
                    if i == 0:
                        # first strip: stream each quarter out as soon as its
                        # copy lands (shortens the cold-start ramp)
                        nc.sync.dma_start(
                            out[0:128, j * 1024:(j + 1) * 1024],
                            ob[:, j * 1024:(j + 1) * 1024],
                        )
                    elif j == 1:
                        # stream each strip's first half out early: the DMA
                        # only waits on two copies, halving the per-strip
                        # issue bubble
                        nc.sync.dma_start(
                            out[i * 128:(i + 1) * 128, :d // 2], ob[:, :d // 2]
                        )
                if i > 0:
                    nc.sync.dma_start(
                        out[i * 128:(i + 1) * 128, d // 2:], ob[:, d // 2:]
                    )
    nc.compile()
    return nc


def _get_kernels():
    if "k1" not in _cache:
        _cache["k1"] = build_k1()
        _cache["k2"] = build_k2()
    return _cache["k1"], _cache["k2"]


def _vt_layout(V, d, r):
    """[128, (d//128)*r] bf16 with vt[p, c*r + j] = V[j, c*128 + p]."""
    nch = d // 128
    # V [r, d] -> [r, nch, 128] -> [128, nch, r]
    return np.ascontiguousarray(
        V.reshape(r, nch, 128).transpose(2, 1, 0).reshape(128, nch * r)
    ).astype(BF16)


def _consts_layout():
    c = np.zeros((128, 768), dtype=np.float32)
    c[:, 0:128] = np.eye(128, dtype=np.float32)
    c[:, 128] = 1.0
    return c.astype(BF16)


def _host_routing(x, core_keys, core_pool, U_shared, cs):
    """cs: list of 8 per-core colsum vectors [d] (f64). Returns per-batch
    wt = (U @ Lam_b)^T as bf16 [R, D]."""
    wt_b = []
    for b in range(B):
        colsum = cs[2 * b] + cs[2 * b + 1]
        mean = colsum / S
        centroid = 0.7 * x[b, -1, :].astype(np.float64) + 0.3 * mean
        c_n = centroid / max(np.linalg.norm(centroid), EPS)
        kk = core_keys.astype(np.float64)
        k_n = kk / np.maximum(np.linalg.norm(kk, axis=-1, keepdims=True), EPS)
        sim = c_n @ k_n.T  # [K]
        logits = sim / TEMP
        e = np.exp(logits - logits.max())
        w = e / e.sum()
        Lam = np.einsum("k,kij->ij", w, core_pool.astype(np.float64))  # [R, R]
        W = U_shared.astype(np.float64) @ Lam  # [D, R]
        wt_b.append(np.ascontiguousarray(W.T).astype(BF16))  # [R, D]
    return wt_b


def kernel(x, V_shared, U_shared, core_pool, core_keys):
    x = np.asarray(x)
    V_shared = np.asarray(V_shared)
    U_shared = np.asarray(U_shared)
    core_pool = np.asarray(core_pool)
    core_keys = np.asarray(core_keys)

    nc1, nc2 = _get_kernels()
    core_ids = list(range(NCORES))

    vt_np = _vt_layout(V_shared.astype(np.float32), D, R)
    consts_np = _consts_layout()

    in_maps1 = []
    for c in core_ids:
        b, h = c // 2, c % 2
        xs = np.ascontiguousarray(x[b, h * SH:(h + 1) * SH, :], dtype=np.float32)
        in_maps1.append({"xs": xs, "vt": vt_np, "consts": consts_np})
    res1 = run_bass_kernel_spmd(nc1, in_maps1, core_ids).results

    # --- host routing (tiny: 16 numbers through softmax) ---
    cs = [res1[c]["cs"].astype(np.float64).ravel() for c in core_ids]  # [d]
    # xvt comes back as 4 col-bands of r rows; sum them
    xvt = [
        res1[c]["xvt"].reshape(4, R, SH).sum(axis=0).astype(np.float32)
        for c in core_ids
    ]  # [r, SH]

    wt_b = _host_routing(x, core_keys, core_pool, U_shared, cs)

    in_maps2 = []
    for c in core_ids:
        b = c // 2
        in_maps2.append({"xvt": xvt[c].astype(BF16), "wt": wt_b[b]})
    res2 = run_bass_kernel_spmd(nc2, in_maps2, core_ids).results

    out = np.empty((B, S, D), dtype=np.float32)
    for c in core_ids:
        b, h = c // 2, c % 2
        out[b, h * SH:(h + 1) * SH, :] = res2[c]["out"].astype(np.float32)
    return out
